# revision 3
# baseline (speedup 1.0000x reference)
"""LocationAwareAttention TRN2 kernel v3 — 8-core SPMD, head+batch sharded.

Sharding: core c handles batch b = c//4 and heads 4*(c%4) .. 4*(c%4)+3.

Changes over the original baseline (all bf16 numerics kept):
- All per-rep DMAs are partition-contiguous with >=8KB per-partition runs
  (measured: DMA cost is ~0.47us per small line; >=8KB lines are ~free):
  x loads as one [128, 8, 2048] tile (32KB/partition), output partials
  store bf16 as [128, 16, 1024] (partition-major) in two 16KB DMAs.
- v^T computed directly as x^T-chunk @ Wv (out = [tokens, vdims]): no PE
  transposes, no identity input, no v bias-add on device (v-bias folded in
  on host as a constant row correction through proj).
- Output partials in bf16 (half the store bytes); host sums in f32.
- x for rep r+1 prefetched during rep r.
- The rep loop is a hardware For_i loop: the NEFF size is constant in
  reps (one body copy), so per-invocation host/tunnel overhead no longer
  grows with the rep count.
"""

import ml_dtypes
import numpy as np

import concourse.bass as bass
import concourse.mybir as mybir
import concourse.tile as tile
from concourse.bass_utils import run_bass_kernel_spmd
from concourse.vector_clock import ScopedClock

B, N, C = 2, 2048, 1024
H, HD = 16, 64
GH = C // 4
P = 128
HPC = 4          # heads per core
N_CORES = 8
NKB = N // P     # 16 key blocks
SCALE = HD ** -0.5

f32 = mybir.dt.float32
f32r = mybir.dt.float32r
bf16 = mybir.dt.bfloat16
AF = mybir.ActivationFunctionType


class SplitDrainTileContext(tile.TileContext):
    """Workaround: this container's walrus rejects >1 sync wait on the Tile
    exit InstDrain ("Too many sync wait commands"). Split the final drain's
    waits across chained single-wait drains."""

    def _drain_and_barrier(self, tick_clock, wait_clock):
        nc = self.nc
        drain_inst = nc.sync.drain()
        wait_clock.add_sem_waits(
            drain_inst.ins, ScopedClock({None: tick_clock.global_clock})
        )
        si = drain_inst.ins.sync_info
        waits = list(si.on_wait) if si and si.on_wait else []
        if len(waits) > 1:
            SyncInfo = type(si)
            drain_inst.ins.sync_info = SyncInfo(
                on_wait=waits[:1], on_update=list(si.on_update)
            )
            for i in range(1, len(waits)):
                extra = nc.sync.drain()
                esi = extra.ins.sync_info
                upd = list(esi.on_update) if esi and esi.on_update else []
                extra.ins.sync_info = SyncInfo(on_wait=waits[i : i + 1], on_update=upd)

        nc.all_engine_barrier()
        assert self.sems is not None
        popped = nc._tile_sem_poison_stack.pop()
        assert popped is self._sem_poison
        nc.clear_and_free_semaphores(list(self.sems.allocated().values()))
        nc.all_engine_barrier()


def split_excess_waits(nc, max_waits=1):
    """This container's walrus rejects instructions carrying more than one
    sync wait. Move excess waits onto same-engine InstNoOps inserted just
    before the instruction (engine streams are in-order, so waiting earlier
    on the same engine is equivalent)."""
    n = 0
    for bb in nc.main_func.blocks:
        il = bb.instructions
        out = []
        changed = False
        for ins in il:
            si = ins.sync_info
            waits = list(si.on_wait) if si and si.on_wait else []
            if len(waits) > max_waits:
                SyncInfo = type(si)
                for i in range(0, len(waits) - max_waits):
                    nop = mybir.InstNoOp(
                        name=f"I-wsplit-{n}",
                        engine=ins.engine,
                        bass_nofuse=True,
                        sync_info=SyncInfo(on_wait=[waits[i]], on_update=[]),
                    )
                    n += 1
                    nc.register_instruction(nop)
                    out.append(nop)
                ins.sync_info = SyncInfo(
                    on_wait=waits[len(waits) - max_waits :],
                    on_update=list(si.on_update),
                )
                changed = True
            out.append(ins)
        if changed:
            bb.instructions = out
    return nc


def build_nc(reps: int = 1):
    nc = bass.Bass()

    xbd = nc.dram_tensor("xbd", [P, 8, N], bf16, kind="ExternalInput")
    wbd = nc.dram_tensor("wbd", [P, 8, 6 * P], bf16, kind="ExternalInput")
    qkbd = nc.dram_tensor("qkbd", [P, 4], f32, kind="ExternalInput")
    pjbd = nc.dram_tensor("pjbd", [P, 2, C], bf16, kind="ExternalInput")
    gmask = nc.dram_tensor("gmask", [P, NKB], f32, kind="ExternalInput")
    g1w_t = nc.dram_tensor("g1w_t", [P, GH], f32, kind="ExternalInput")
    g1b_t = nc.dram_tensor("g1b_t", [P, GH], f32, kind="ExternalInput")
    g2w_t = nc.dram_tensor("g2w_t", [P, GH], f32, kind="ExternalInput")
    g2b_t = nc.dram_tensor("g2b_t", [P, 1], f32, kind="ExternalInput")
    outp = nc.dram_tensor("outp", [P, NKB, C], bf16, kind="ExternalOutput")

    with SplitDrainTileContext(nc) as tc:
        with (
            tc.tile_pool(name="const", bufs=1) as const,
            tc.tile_pool(name="xin", bufs=2) as xin,
            tc.tile_pool(name="qkt", bufs=1) as qkt,
            tc.tile_pool(name="vext", bufs=1) as vextp,
            tc.tile_pool(name="exps", bufs=21) as expsp,
            tc.tile_pool(name="outt", bufs=1) as outtp,
            tc.tile_pool(name="small", bufs=4) as small,
            tc.tile_pool(name="obig", bufs=1) as obigp,
            tc.tile_pool(name="gates", bufs=2) as gatesp,
            tc.tile_pool(name="mm", bufs=2, space="PSUM") as mm,
            tc.tile_pool(name="pso", bufs=3, space="PSUM") as pso,
            tc.tile_pool(name="recb", bufs=1, space="PSUM") as recbp,
        ):
            # ---- constant loads ----
            wb = const.tile([P, 8, 6 * P], bf16, tag="wb")
            nc.sync.dma_start(wb[:], wbd[:])
            qkb = const.tile([P, 4], f32, tag="qkb")
            nc.sync.dma_start(qkb[:], qkbd[:])
            pjb = const.tile([P, 2, C], bf16, tag="pjb")
            nc.sync.dma_start(pjb[:], pjbd[:])
            gm_sb = const.tile([P, NKB], f32, tag="gm")
            nc.sync.dma_start(gm_sb[:], gmask[:])
            g1w_sb = const.tile([P, GH], f32, tag="g1w")
            nc.sync.dma_start(g1w_sb[:], g1w_t[:])
            g1b_sb = const.tile([P, GH], f32, tag="g1b")
            nc.sync.dma_start(g1b_sb[:], g1b_t[:])
            g2w_sb = const.tile([P, GH], f32, tag="g2w")
            nc.sync.dma_start(g2w_sb[:], g2w_t[:])
            g2b_sb = const.tile([P, 1], f32, tag="g2b")
            nc.sync.dma_start(g2b_sb[:], g2b_t[:])
            ones_col = const.tile([1, 64], f32, tag="ones_col")
            nc.vector.memset(ones_col[:], 1.0)

            xb = xin.tile([P, 8, N], bf16, tag="xb", name="xb0")
            nc.sync.dma_start(xb[:], xbd[:])

            with tc.For_i(0, reps) as _i:

                # ---- spatial gate: gatesc[k] = SCALE * sigmoid(mlp(mask[k])) ----
                gatesc = gatesp.tile([P, NKB], f32, tag="gatesc")
                for kb in range(NKB):
                    m_col = gm_sb[:, kb : kb + 1]
                    t1 = gatesp.tile([P, GH], f32, tag="g_t1")
                    nc.vector.tensor_scalar_mul(t1[:], g1w_sb[:], m_col)
                    nc.vector.tensor_add(t1[:], t1[:], g1b_sb[:])
                    nc.scalar.activation(t1[:], t1[:], AF.Relu)
                    nc.vector.tensor_mul(t1[:], t1[:], g2w_sb[:])
                    gp = gatesp.tile([P, 1], f32, tag="g_gp")
                    nc.vector.reduce_sum(gp[:], t1[:], axis=mybir.AxisListType.X)
                    nc.scalar.activation(gp[:], gp[:], AF.Sigmoid, bias=g2b_sb[:])
                    nc.scalar.mul(gatesc[:, kb : kb + 1], gp[:], SCALE)

                # ---- q,k tiles: qk[mi] = [128 dims, 2048 tok] bf16, mi in
                #      {q01, q23, k01, k23} (m col offset mi*128 in wb) ----
                qk = {
                    mi: qkt.tile([P, N], bf16, tag=f"qk{mi}", name=f"qk{mi}")
                    for mi in range(4)
                }

                def qkv_chain(mi, nf):
                    ps = mm.tile([P, 1024], f32, tag="mmt", name="qkv_ps")
                    for kc in range(8):
                        for half in range(2):
                            off = nf * 1024 + half * 512
                            nc.tensor.matmul(
                                ps[:, half * 512 : (half + 1) * 512],
                                lhsT=wb[:, kc, mi * P : (mi + 1) * P],
                                rhs=xb[:, kc, off : off + 512],
                                start=(kc == 0),
                                stop=(kc == 7),
                            )
                    nc.vector.tensor_scalar_add(
                        qk[mi][:, nf * 1024 : (nf + 1) * 1024], ps[:],
                        qkb[:, mi : mi + 1],
                    )

                # q01 + k01 first: group 0's S^T can start (feeding ACT) while
                # the rest of qkv and v^T still stream on PE.
                for mi in (0, 2):
                    for nf in range(2):
                        qkv_chain(mi, nf)

                vext = vextp.tile([P, HPC, NKB, 65], bf16, tag="vext")
                nc.vector.memset(vext[:, :, :, 64:65], 1.0)

                def vt_chain(tb):
                    # v^T psums go to the pso pool (idle until the group loop)
                    # so the mm pool stays dedicated to S^T/qkv staging.
                    ps = pso.tile([P, 512], f32, tag="pso", name="vt_ps")
                    for kc in range(8):
                        nc.tensor.matmul(
                            ps[:, 0:256],
                            lhsT=xb[:, kc, tb * P : (tb + 1) * P],
                            rhs=wb[:, kc, 4 * P : 6 * P],
                            start=(kc == 0),
                            stop=(kc == 7),
                        )
                    for h in range(HPC):
                        nc.vector.tensor_copy(
                            vext[:, h, tb, 0:64], ps[:, h * 64 : h * 64 + 64]
                        )

                # ---- attention: software-pipelined over 8 (qh, hp) groups ----
                outT = outtp.tile([P, 2, N], bf16, tag="outT")
                groups = [(qh, hp) for qh in range(4) for hp in range(2)]

                def st_exp(gi, kb):
                    qh, hp = groups[gi]
                    qm, km = hp, 2 + hp
                    qs = qh * 512
                    ps = mm.tile([P, 1024], f32, tag="mmt", name="st_ps")
                    for par in range(2):
                        nc.tensor.matmul(
                            ps[:, par * 512 : (par + 1) * 512],
                            lhsT=qk[km][par * 64 : par * 64 + 64, kb * P : (kb + 1) * P],
                            rhs=qk[qm][par * 64 : par * 64 + 64, qs : qs + 512],
                            start=True,
                            stop=True,
                        )
                    e = expsp.tile([P, 1024], bf16, tag="exps", name="exps_t")
                    nc.scalar.activation(e[:], ps[:], AF.Exp, scale=gatesc[:, kb : kb + 1])
                    return e

                # prologue: group-0 S^T/exp interleaved with v^T and the
                # remaining qkv chains, so ACT ramps while PE streams.
                qkv_rest = [(1, 0), (1, 1), (3, 0), (3, 1)]
                exps_cur = []
                for kb in range(NKB):
                    exps_cur.append(st_exp(0, kb))
                    vt_chain(kb)
                    if kb < len(qkv_rest):
                        qkv_chain(*qkv_rest[kb])

                # refill x for the next iteration (same data; keeps x
                # streaming honest); overlaps the attention stage below.
                nc.sync.dma_start(xb[:], xbd[:])

                for gi in range(len(groups)):
                    qh, hp = groups[gi]
                    qs = qh * 512
                    ps_os = [pso.tile([P, 512], f32, tag="pso", name="pso_t") for _ in range(2)]
                    exps_next = []
                    for kb in range(NKB):
                        if gi + 1 < len(groups):
                            exps_next.append(st_exp(gi + 1, kb))
                        for par in range(2):
                            nc.tensor.matmul(
                                ps_os[par][0:65, :],
                                lhsT=vext[:, 2 * hp + par, kb, :],
                                rhs=exps_cur[kb][:, par * 512 : (par + 1) * 512],
                                start=(kb == 0),
                                stop=(kb == NKB - 1),
                            )
                    for par in range(2):
                        ps_o = ps_os[par]
                        rec = small.tile([1, 512], f32r, tag="rec")
                        with nc.allow_low_precision(reason="denominator reciprocal at tf32 precision"):
                            nc.vector.reciprocal(rec[:], ps_o[64:65, :])
                        rb = recbp.tile([64, 512], f32, tag="recb")
                        nc.tensor.matmul(
                            rb[:], lhsT=ones_col[:].bitcast(f32r), rhs=rec[:],
                            start=True, stop=True,
                        )
                        rb_sb = small.tile([64, 512], f32, tag="recb_sb")
                        nc.vector.tensor_copy(rb_sb[:], rb[:])
                        nc.vector.tensor_mul(
                            outT[par * 64 : par * 64 + 64, hp, qs : qs + 512],
                            ps_o[0:64, :],
                            rb_sb[:],
                        )
                    exps_cur = exps_next
                    if hp == 1:
                        if qh == 0:
                            o_big = obigp.tile([P, NKB, C], bf16, tag="obig")
                        for qc in range(4 * qh, 4 * qh + 4):
                            for cb in range(2):
                                ps = pso.tile([P, 512], f32, tag="pso", name="proj_ps")
                                for j in range(2):
                                    nc.tensor.matmul(
                                        ps[:],
                                        lhsT=outT[:, j, qc * P : (qc + 1) * P],
                                        rhs=pjb[:, j, cb * 512 : (cb + 1) * 512],
                                        start=(j == 0),
                                        stop=(j == 1),
                                    )
                                nc.vector.tensor_copy(
                                    o_big[:, qc, cb * 512 : (cb + 1) * 512], ps[:]
                                )
                        if qh == 1:
                            nc.sync.dma_start(outp[:, 0:8, :], o_big[:, 0:8, :])
                        elif qh == 3:
                            nc.sync.dma_start(outp[:, 8:16, :], o_big[:, 8:16, :])

    return split_excess_waits(nc)


def shard_inputs(x, spatial_mask, qkv_w, qkv_b, proj_w, g1_w, g1_b, g2_w, g2_b):
    in_maps = []
    for c in range(N_CORES):
        b = c // (N_CORES // B)
        heads = [HPC * (c % (N_CORES // B)) + i for i in range(HPC)]
        dsel = np.array([h * HD + j for h in heads for j in range(HD)])
        sel = np.concatenate([dsel, C + dsel, 2 * C + dsel])

        # xbd[p, kc, n] = x[b, n, kc*128+p]
        xbd = np.ascontiguousarray(
            x[b].T.reshape(8, P, N).transpose(1, 0, 2)
        ).astype(ml_dtypes.bfloat16)
        # wbd[p, kc, m] = qkv_w[sel[m], kc*128+p]
        wbd = np.ascontiguousarray(
            qkv_w[sel, :].T.reshape(8, P, 6 * P).transpose(1, 0, 2)
        ).astype(ml_dtypes.bfloat16)
        # q,k biases per m-tile (tiles 0..3 of sel are the q,k dims)
        qkbd = np.ascontiguousarray(
            qkv_b[sel[: 4 * P]].reshape(4, P).T
        ).astype(np.float32)
        # pjbd[p, j, cc] = proj_w[cc, dsel[j*128+p]]
        pjbd = np.ascontiguousarray(
            proj_w[:, dsel].T.reshape(2, P, C).transpose(1, 0, 2)
        ).astype(ml_dtypes.bfloat16)

        in_maps.append(
            {
                "xbd": xbd,
                "wbd": wbd,
                "qkbd": qkbd,
                "pjbd": pjbd,
                "gmask": np.ascontiguousarray(spatial_mask[b].reshape(NKB, P).T),
                "g1w_t": np.ascontiguousarray(np.tile(g1_w[:, 0][None, :], (P, 1))),
                "g1b_t": np.ascontiguousarray(np.tile(g1_b[None, :], (P, 1))),
                "g2w_t": np.ascontiguousarray(np.tile(g2_w[0][None, :], (P, 1))),
                "g2b_t": np.full((P, 1), g2_b[0], dtype=np.float32),
            }
        )
    return in_maps


_NC_CACHE = None


def kernel(x, spatial_mask, qkv_w, qkv_b, proj_w, proj_b, g1_w, g1_b, g2_w, g2_b):
    global _NC_CACHE
    x = np.asarray(x, dtype=np.float32)
    spatial_mask = np.asarray(spatial_mask, dtype=np.float32)
    qkv_w = np.asarray(qkv_w, dtype=np.float32)
    qkv_b = np.asarray(qkv_b, dtype=np.float32)
    proj_w = np.asarray(proj_w, dtype=np.float32)
    proj_b = np.asarray(proj_b, dtype=np.float32)
    g1_w = np.asarray(g1_w, dtype=np.float32)
    g1_b = np.asarray(g1_b, dtype=np.float32)
    g2_w = np.asarray(g2_w, dtype=np.float32)
    g2_b = np.asarray(g2_b, dtype=np.float32)

    if _NC_CACHE is None:
        _NC_CACHE = build_nc()
    nc = _NC_CACHE
    in_maps = shard_inputs(
        x, spatial_mask, qkv_w, qkv_b, proj_w, g1_w, g1_b, g2_w, g2_b
    )
    res = run_bass_kernel_spmd(nc, in_maps, list(range(N_CORES)))
    # outp[p, qc, cc] -> partial[qc*128+p, cc]
    parts = [
        np.asarray(res.results[c]["outp"], dtype=np.float32)
        .transpose(1, 0, 2)
        .reshape(N, C)
        for c in range(N_CORES)
    ]
    cpb = N_CORES // B
    full = np.stack([sum(parts[b * cpb : (b + 1) * cpb]) for b in range(B)])
    # v-bias correction (v bias is not applied on device): out += b_v @ W_p^T
    vb = qkv_b[2 * C : 3 * C]
    corr = vb @ proj_w.T + proj_b
    return (full + corr[None, None, :]).astype(np.float32)
